# revision 11
# baseline (speedup 1.0000x reference)
"""Multi-head attention Trainium2 kernel (8 NeuronCores, SPMD).

Problem: B=2, S=2048, E=1024, H=16, D=64 causal MHA with fp32 reference.

Sharding: core c handles batch b = c // 4 and heads [4*(c%4), 4*(c%4)+4).
Each core computes its 4 heads' Q/K/V projections, causal attention, and a
partial output projection against its rows of Wp.  The host sums the four
partials per batch and adds the bias.

Design (v2, software-pipelined):
  - The Scalar (ACT) engine is the attention-phase floor (~95 G elem/s on
    exp), so it runs ONLY the softmax EXPs.  All PSUM drains / normalize
    multiplies run on DVE; mask triangles + memsets on GpSimd.
  - The PE schedule is a 4-super-step pipeline over q-tiles: attention
    wave qi interleaves the projections for step qi+1 and the output
    projection for step qi-1, keeping the PE dense so its pstate (and the
    HW HAM clock gate) stays warm.
  - EXP work is trimmed per causal block: fully-masked column ranges are
    memset to zero instead of exp'd; the diagonal 128x128 triangle is
    masked with one constant triu tile (derived from the actual mask at
    build time; arbitrary masks fall back to a general masked path).
  - Softmax denominator comes free from a ones-column appended to V; the
    reciprocal uses the fast approx DVE op (5x faster), broadcast across
    partitions with a K=1 f32r ones matmul.
  - Output projection pairs heads (K=128); the second head-pair's
    normalized tile is shifted to partitions 64-127 via a tiny SBUF->SBUF
    DMA (DVE is lane-locked and cannot shift partitions).
  - Inputs stream in four 512-column chunks so the first matmul can start
    at ~4us; a short burst of warm-up matmuls on memset data covers the
    DMA window to pre-warm the PE pstate.
  - Output is written bf16 (halves the output DMA); host sums in fp32.
"""

import sys
from collections import deque

import numpy as np

sys.path.insert(0, "/opt/trn_rl_repo")

import ml_dtypes  # noqa: E402
import concourse.bass as bass  # noqa: E402,F401
import concourse.tile as tile  # noqa: E402
from concourse import bacc, mybir  # noqa: E402
from concourse.bass_utils import run_bass_kernel_spmd  # noqa: E402

F32 = mybir.dt.float32
F32R = mybir.dt.float32r
BF16 = mybir.dt.bfloat16
EXP = mybir.ActivationFunctionType.Exp
COPY = mybir.ActivationFunctionType.Copy
BF = ml_dtypes.bfloat16

B, S, E, H, D = 2, 2048, 1024, 16, 64
N_CORES = 8
HC = H // 4          # heads per core (4)
EC = HC * D          # head cols per core (256)
QT = 512             # query tile (free dim of score matmuls)
KT = 128             # key tile (partition dim of score tiles)

FULL, DIAG, GEN = 0, 1, 2


def build_program(schedule, n_general=0):
    """Build the per-core Bass program.

    schedule: list over q-tiles of lists of (kj, kind, qlo, gidx).
    """
    nq = S // QT     # 4
    nkc = E // 128   # contraction tiles for projections (8)
    nm = S // 128    # m-tiles for V / output (16)

    nc = bacc.Bacc(None, target_bir_lowering=False, debug=False)

    xqT = nc.dram_tensor("xqT", [E, S], BF16, kind="ExternalInput")
    xkT = nc.dram_tensor("xkT", [E, S], BF16, kind="ExternalInput")
    xvT = nc.dram_tensor("xvT", [E, S], BF16, kind="ExternalInput")
    wq = nc.dram_tensor("wq", [E, EC], BF16, kind="ExternalInput")
    wk = nc.dram_tensor("wk", [E, EC], BF16, kind="ExternalInput")
    wv = nc.dram_tensor("wv", [E, EC], BF16, kind="ExternalInput")
    wp = nc.dram_tensor("wp", [EC, E], BF16, kind="ExternalInput")
    tri = nc.dram_tensor("tri", [KT, KT], BF16, kind="ExternalInput")
    mtd = None
    if n_general:
        mtd = nc.dram_tensor("mtd", [n_general * KT, QT], BF16,
                             kind="ExternalInput")
    outp = nc.dram_tensor("outp", [S, E], BF16, kind="ExternalOutput")

    with tile.TileContext(nc) as tc:
        with (
            tc.tile_pool(name="const", bufs=1) as const,
            tc.tile_pool(name="big", bufs=1) as big,
            tc.tile_pool(name="pt", bufs=4) as ptp,
            tc.tile_pool(name="rd", bufs=2) as rdp,
            tc.tile_pool(name="bc", bufs=2) as bcp,
            tc.tile_pool(name="ott", bufs=2) as otp,
            tc.tile_pool(name="osb", bufs=2) as osbp,
            tc.tile_pool(name="ps", bufs=1, space="PSUM") as psp,
        ):
            # ---- persistent SBUF ----
            wq_sb = const.tile([128, nkc, EC], BF16, tag="wq")
            wk_sb = const.tile([128, nkc, EC], BF16, tag="wk")
            wv_sb = const.tile([128, nkc, EC], BF16, tag="wv")
            wp2_sb = [const.tile([128, E], BF16, tag=f"wp{g}",
                                 name=f"wp2_sb{g}")
                      for g in range(2)]
            tri_sb = const.tile([128, KT], BF16, tag="tri")
            xfq = const.tile([128, nkc, S], BF16, tag="xfq")
            xfk = const.tile([128, nkc, S], BF16, tag="xfk")
            xfv = const.tile([128, nkc, S], BF16, tag="xfv")
            ones_f = const.tile([128, 64], F32, tag="onesf")
            ones_r = const.tile([128, 64], F32R, tag="onesr")
            warm_sb = const.tile([128, 128], BF16, tag="warm")
            mt_sb = None
            if n_general:
                mt_sb = const.tile([128, n_general, QT], BF16, tag="mt")

            QTg = [big.tile([128, S], BF16, tag=f"qt{g}", name=f"QTg{g}")
                   for g in range(2)]
            KTg = [big.tile([128, S], BF16, tag=f"kt{g}", name=f"KTg{g}")
                   for g in range(2)]
            vaug = big.tile([128, nm, HC, 65], BF16, tag="vaug")
            OT2 = [big.tile([128, S], BF16, tag=f"ot{g}", name=f"OT2_{g}")
                   for g in range(2)]

            # ---- input DMAs ----
            # First half (cols 0-1023) lands per-kc with 2KB descriptors so
            # the projection chains can start as soon as their rows arrive.
            # The second half is issued later (from the gpsimd stream, once
            # attention is underway) so it doesn't steal bandwidth from the
            # critical first half.
            HS = S // 2
            nc.sync.dma_start(out=wq_sb,
                              in_=wq.rearrange("(kc p) n -> p kc n", p=128))
            for kc in range(nkc):
                nc.sync.dma_start(out=xfq[:, kc, 0:HS],
                                  in_=xqT[kc * 128:(kc + 1) * 128, 0:HS])
            nc.sync.dma_start(out=wk_sb,
                              in_=wk.rearrange("(kc p) n -> p kc n", p=128))
            nc.scalar.dma_start(out=wv_sb,
                                in_=wv.rearrange("(kc p) n -> p kc n",
                                                 p=128))
            for kc in range(nkc):
                nc.scalar.dma_start(out=xfk[:, kc, 0:HS],
                                    in_=xkT[kc * 128:(kc + 1) * 128, 0:HS])
            for kc in range(nkc):
                nc.gpsimd.dma_start(out=xfv[:, kc, 0:HS],
                                    in_=xvT[kc * 128:(kc + 1) * 128, 0:HS])
            nc.sync.dma_start(out=tri_sb, in_=tri[:, :])
            for g in range(2):
                nc.sync.dma_start(out=wp2_sb[g],
                                  in_=wp[g * 128:(g + 1) * 128, :])
            if n_general:
                nc.sync.dma_start(
                    out=mt_sb, in_=mtd.rearrange("(t p) q -> p t q", p=KT))

            def issue_second_half():
                xqr = xqT.rearrange("(kc p) s -> p kc s", p=128)
                xkr = xkT.rearrange("(kc p) s -> p kc s", p=128)
                xvr = xvT.rearrange("(kc p) s -> p kc s", p=128)
                hs = slice(HS, S)
                nc.gpsimd.dma_start(out=xfq[:, :, hs], in_=xqr[:, :, hs])
                nc.gpsimd.dma_start(out=xfk[:, :, hs], in_=xkr[:, :, hs])
                nc.gpsimd.dma_start(out=xfv[:, :, hs], in_=xvr[:, :, hs])

            # ---- constants + PE warm-up (no DMA dependency) ----
            nc.vector.memset(ones_f, 1.0)
            nc.vector.tensor_copy(ones_r, ones_f)
            nc.vector.memset(warm_sb, 1.0)
            for _ in range(14):
                wps = psp.tile([128, 512], F32, tag="misc", bufs=2)
                nc.tensor.matmul(wps[:, 0:128], warm_sb, warm_sb,
                                 start=True, stop=True)

            # ---- projection / output-projection emitters ----
            def qk_chain(dst, w_sb, xf, g, mt, drain_eng):
                ps = psp.tile([128, 512], F32, tag="misc", bufs=2)
                for kc in range(nkc):
                    nc.tensor.matmul(
                        ps, w_sb[:, kc, 128 * g:128 * (g + 1)],
                        xf[:, kc, mt * QT:(mt + 1) * QT],
                        start=(kc == 0), stop=(kc == nkc - 1))
                if drain_eng == "scalar":
                    nc.scalar.activation(
                        dst[:, mt * QT:(mt + 1) * QT], ps, COPY)
                else:
                    nc.vector.tensor_copy(dst[:, mt * QT:(mt + 1) * QT], ps)

            def v_chain(mt, drain_eng):
                ps = psp.tile([128, 512], F32, tag="misc", bufs=2)
                psv = ps[:, 0:EC]
                for kc in range(nkc):
                    nc.tensor.matmul(
                        psv, xfv[:, kc, mt * 128:(mt + 1) * 128],
                        wv_sb[:, kc, :],
                        start=(kc == 0), stop=(kc == nkc - 1))
                dst = vaug[:, mt, :, 0:64]
                src = psv.rearrange("p (h d) -> p h d", h=HC)
                if drain_eng == "scalar":
                    nc.scalar.activation(dst, src, COPY)
                else:
                    nc.vector.tensor_copy(dst, src)
                nc.gpsimd.memset(vaug[:, mt, :, 64], 1.0)

            def proj_ops(qi, drain_eng):
                ops = []
                for g in range(2):
                    ops.append(lambda g=g: qk_chain(
                        QTg[g], wq_sb, xfq, g, qi, drain_eng))
                for g in range(2):
                    ops.append(lambda g=g: qk_chain(
                        KTg[g], wk_sb, xfk, g, qi, drain_eng))
                for mt in range(4 * qi, 4 * qi + 4):
                    ops.append(lambda mt=mt: v_chain(mt, drain_eng))
                return ops

            def outproj(mt):
                osb = osbp.tile([128, 1024], BF16, tag="osb")
                for et in range(2):
                    ps = psp.tile([128, 512], F32, tag="misc", bufs=2)
                    for g in range(2):
                        nc.tensor.matmul(
                            ps, OT2[g][:, mt * 128:(mt + 1) * 128],
                            wp2_sb[g][:, et * 512:(et + 1) * 512],
                            start=(g == 0), stop=(g == 1))
                    nc.vector.tensor_copy(osb[:, et * 512:(et + 1) * 512],
                                          ps)
                nc.sync.dma_start(out=outp[mt * 128:(mt + 1) * 128, :],
                                  in_=osb)

            def outproj_ops(qi):
                return [lambda mt=mt: outproj(mt)
                        for mt in range(4 * qi, 4 * qi + 4)]

            # ---- pipelined main loop ----
            pending = deque()

            def pop_filler(blocks_left):
                n = -(-len(pending) // max(blocks_left, 1))
                for _ in range(min(n, len(pending))):
                    pending.popleft()()

            for op in proj_ops(0, "scalar"):
                op()

            second_half_issued = False
            for qi in range(nq):
                ks = schedule[qi]
                if qi + 1 < nq:
                    pending.extend(proj_ops(qi + 1, "vector"))
                if qi == nq - 1:
                    for q2 in range(nq - 1):
                        pending.extend(outproj_ops(q2))
                nblk = 2 * len(ks)
                for g in range(2):
                    acc = psp.tile([65, 2, 512], F32, tag="acc", bufs=1)
                    for idx, (kj, kind, qlo, gidx) in enumerate(ks):
                        stp = psp.tile([128, 2, 512], F32, tag="stp",
                                       bufs=2)
                        for s in range(2):
                            base = 64 * s
                            nc.tensor.matmul(
                                stp[:, s, :],
                                KTg[g][base:base + 64,
                                       kj * KT:(kj + 1) * KT],
                                QTg[g][base:base + 64,
                                       qi * QT:(qi + 1) * QT],
                                start=True, stop=True)
                        ptw = ptp.tile([128, 2, 512], BF16, tag="pt")
                        for s in range(2):
                            if kind == DIAG:
                                nc.scalar.activation(
                                    ptw[:, s, qlo:QT], stp[:, s, qlo:QT],
                                    EXP, scale=0.125)
                                if qlo:
                                    nc.gpsimd.memset(ptw[:, s, 0:qlo], 0.0)
                                nc.gpsimd.tensor_mul(
                                    ptw[:, s, qlo:qlo + KT],
                                    ptw[:, s, qlo:qlo + KT], tri_sb)
                            else:
                                nc.scalar.activation(
                                    ptw[:, s, :], stp[:, s, :],
                                    EXP, scale=0.125)
                                if kind == GEN:
                                    nc.vector.tensor_mul(
                                        ptw[:, s, :], ptw[:, s, :],
                                        mt_sb[:, gidx, :])
                            nc.tensor.matmul(
                                acc[:, s, :], vaug[:, kj, 2 * g + s, :],
                                ptw[:, s, :],
                                start=(idx == 0), stop=(idx == len(ks) - 1))
                        nblk -= 1
                        if not second_half_issued:
                            issue_second_half()
                            second_half_issued = True
                        pop_filler(nblk)
                    # normalize this head-pair wave
                    for s in range(2):
                        # round-copy raw denom to f32r (partition-safe),
                        # broadcast via K=1 matmul, then fast-reciprocal at
                        # base partition 0 (the custom DVE op is broken at
                        # non-zero base partitions)
                        rdr = rdp.tile([65, 512], F32R, tag="rdr")
                        nc.vector.tensor_copy(rdr[64:65, :],
                                              acc[64:65, s, :])
                        bc_ps = psp.tile([128, 512], F32, tag="misc",
                                         bufs=2)
                        nc.tensor.matmul(
                            bc_ps[0:64, :], ones_r[64:65, :],
                            rdr[64:65, :],
                            start=True, stop=True)
                        bc_sb = bcp.tile([64, 512], F32, tag="bc")
                        nc.vector.reciprocal_approx_fast(
                            bc_sb, bc_ps[0:64, :])
                        cols = slice(qi * QT, (qi + 1) * QT)
                        if s == 0:
                            nc.vector.tensor_mul(
                                OT2[g][0:64, cols], acc[0:64, s, :], bc_sb)
                        else:
                            ott = otp.tile([64, 512], BF16, tag="ott")
                            nc.vector.tensor_mul(
                                ott, acc[0:64, s, :], bc_sb)
                            nc.gpsimd.dma_start(
                                out=OT2[g][64:128, cols], in_=ott)
                while pending:
                    pending.popleft()()

            for op in outproj_ops(nq - 1):
                op()

    nc.compile()
    return nc


def build_schedule(mask):
    """Classify (q-tile, k-tile) blocks from the actual mask content.

    Returns (schedule, mask_blocks): schedule[qi] is a list of
    (kj, kind, qlo, gidx); mask_blocks stacks transposed bf16 masks for
    GEN blocks, shape (n_general*KT, QT).
    """
    nq, nk = S // QT, S // KT
    qidx = np.arange(QT)[:, None]
    kidx = np.arange(KT)[None, :]
    schedule = []
    blocks = []
    for qi in range(nq):
        row = []
        for kj in range(nk):
            sub = mask[qi * QT:(qi + 1) * QT, kj * KT:(kj + 1) * KT]
            if not sub.any():
                continue
            if sub.all():
                row.append((kj, FULL, 0, -1))
                continue
            qlo = KT * (kj - 4 * qi)
            if 0 <= qlo <= QT - KT and np.array_equal(
                    sub, qidx >= qlo + kidx):
                row.append((kj, DIAG, qlo, -1))
            else:
                row.append((kj, GEN, 0, len(blocks)))
                blocks.append(np.ascontiguousarray(sub.T).astype(BF))
        schedule.append(row)
    mask_blocks = (np.concatenate(blocks, axis=0) if blocks
                   else np.zeros((0, QT), BF))
    return schedule, mask_blocks


_CACHE = {}


def _get_program(sched_key, n_general):
    if sched_key not in _CACHE:
        sched = [list(row) for row in sched_key]
        _CACHE[sched_key] = build_program(sched, n_general=n_general)
    return _CACHE[sched_key]


def kernel(xq, xk, xv, Wq, Wk, Wv, Wp, bp, mask, _trace=False):
    xq = np.asarray(xq, np.float32)
    xk = np.asarray(xk, np.float32)
    xv = np.asarray(xv, np.float32)
    Wq = np.asarray(Wq, np.float32)
    Wk = np.asarray(Wk, np.float32)
    Wv = np.asarray(Wv, np.float32)
    Wp = np.asarray(Wp, np.float32)
    bp = np.asarray(bp, np.float32)
    mask = np.asarray(mask)

    schedule, mask_blocks = build_schedule(mask)
    n_general = mask_blocks.shape[0] // KT
    sched_key = tuple(tuple(row) for row in schedule)
    nc = _get_program(sched_key, n_general)

    tri_np = np.ascontiguousarray(
        np.triu(np.ones((KT, KT), np.float32))).astype(BF)

    xT = {}
    for b in range(B):
        xT[("q", b)] = np.ascontiguousarray(xq[b].T).astype(BF)
        xT[("k", b)] = np.ascontiguousarray(xk[b].T).astype(BF)
        xT[("v", b)] = np.ascontiguousarray(xv[b].T).astype(BF)

    in_maps = []
    for c in range(N_CORES):
        b, hg = c // 4, c % 4
        cols = slice(EC * hg, EC * (hg + 1))
        m = {
            "xqT": xT[("q", b)],
            "xkT": xT[("k", b)],
            "xvT": xT[("v", b)],
            "wq": np.ascontiguousarray(Wq[:, cols]).astype(BF),
            "wk": np.ascontiguousarray(Wk[:, cols]).astype(BF),
            "wv": np.ascontiguousarray(Wv[:, cols]).astype(BF),
            "wp": np.ascontiguousarray(Wp[cols, :]).astype(BF),
            "tri": tri_np,
        }
        if n_general:
            m["mtd"] = mask_blocks
        in_maps.append(m)

    res = run_bass_kernel_spmd(nc, in_maps, core_ids=list(range(N_CORES)),
                               trace=_trace)
    out = np.zeros((B, S, E), np.float32)
    for c in range(N_CORES):
        out[c // 4] += np.asarray(res.results[c]["outp"], np.float32)
    out += bp
    if _trace:
        kernel._last_results = res
    return out
